# revision 40
# baseline (speedup 1.0000x reference)
"""Trainium2 Bass kernel for nn_BiologicalMemory (retrieval_knn).

Computes: q = mean(query, axis=0); sims = cosine(bank, q); i* = argmax(sims);
out = (sims[i*] > 0.65) ? bank[i*] @ w_dec.T + b_dec : zeros.

Strategy (8 NeuronCores, SPMD), filter-then-rerank:
  - bank rows sharded 16384/core. The similarity SEARCH streams an fp8-e4m3
    host-transposed copy of the bank (16 MB/core, 4 KB DMA lines) through the
    PE as DoubleRow matvecs against q (fp8): 512-row dot blocks accumulate in
    PSUM over 4 chunk-pair matmuls (256-dim contraction each).
  - q = column sums of the replicated bf16 query, accumulated on the PE with
    a ones-vector lhsT (fp32 PSUM accumulation).
  - block dots are copied to SBUF, DMA-round-tripped into a [128,128] layout,
    and each partition's top row (by fp8 dot) becomes a rerank candidate.
  - rerank: indirect-gather the 128 candidate rows in fp32, compute exact
    dots vs fp32 q and exact squared norms, score f = dot*|dot|/||x||^2
    (monotone in cosine), and pick the local winner exactly.
  - AllGather 8 candidate records [score, 1.0, row(fp32)]; winner selected by
    score; its row (and the threshold indicator, via the 1.0 marker column)
    is broadcast across partitions with a rank-1 PE matmul; decode is exact
    fp32: out = w_shard . (ind*row) + ind*b_shard per core (128 features).
"""

import os
import sys

import numpy as np

for _p in ("/opt/trn_rl_repo",):
    if os.path.isdir(_p) and _p not in sys.path:
        sys.path.insert(0, _p)

from contextlib import ExitStack

import ml_dtypes
import concourse.bass as bass
import concourse.tile as tile
from concourse import mybir
from concourse.bass_utils import run_bass_kernel_spmd

N_CORES = 8
SEQ, DIM, N_MEM = 2048, 1024, 131072
ROWS_PC = N_MEM // N_CORES  # 16384 bank rows per core
WROWS_PC = DIM // N_CORES  # 128 decoder rows per core
P = 128
NCP = 4  # chunk-pairs (256 dims contracted per DoubleRow matmul)
NB = 32  # 512-row dot blocks per core
BS = 512  # rows per block
NJ = 4  # jb groups (8 blocks each); tile free = 4096 rows
JROWS = ROWS_PC // NJ  # 4096
Q_TILES = SEQ // P  # 16
THR2 = 0.65 * 0.65
Q8_SCALE = 1.0 / 16.0
CW = 2 + DIM  # record: [score, marker=1.0, row...]

F32 = mybir.dt.float32
BF16 = mybir.dt.bfloat16
F8 = mybir.dt.float8e4
U32 = mybir.dt.uint32
AX = mybir.AxisListType
OP = mybir.AluOpType
AF = mybir.ActivationFunctionType
PM = mybir.MatmulPerfMode

_MAX_WAITS = 1


def _split_multi_waits(nc, max_waits=_MAX_WAITS):
    """This walrus build accepts at most one sync-wait per instruction.
    Hoist extra waits onto injected same-engine Drain instructions placed
    immediately before the over-subscribed instruction."""
    counter = 0
    for f in nc.m.functions:
        for bb in f.blocks:
            insts = list(bb.instructions)
            out = []
            changed = False
            for inst in insts:
                si = getattr(inst, "sync_info", None)
                waits = list(si.on_wait) if (si is not None and si.on_wait) else []
                if len(waits) > max_waits:
                    changed = True
                    extra, keep = waits[:-max_waits], waits[-max_waits:]
                    for w in extra:
                        counter += 1
                        d = mybir.InstDrain(name=f"waitsplit-{counter}")
                        d.engine = inst.engine
                        d.sync_info = mybir.SyncInfo(on_wait=[w], on_update=[])
                        out.append(d)
                    inst.sync_info = mybir.SyncInfo(
                        on_wait=keep, on_update=list(si.on_update or [])
                    )
                out.append(inst)
            if changed:
                bb.instructions = out


def _bcast_ap(handle, offset, nparts, nfree):
    """DRAM AP that replicates a contiguous [nfree] region across nparts."""
    return bass.AP(tensor=handle, offset=offset, ap=[[0, nparts], [1, nfree]])


def build_kernel():
    nc = bass.Bass(num_devices=N_CORES)

    bank8 = nc.dram_tensor("bank_t8", [NCP, P, 2, ROWS_PC], F8, kind="ExternalInput")
    bankf = nc.dram_tensor("bank_f32", [ROWS_PC, DIM], F32, kind="ExternalInput")
    qry8 = nc.dram_tensor("query_f8", [SEQ // 256, P, 2, DIM], F8, kind="ExternalInput")
    wsh = nc.dram_tensor("w_shard", [P, DIM], F32, kind="ExternalInput")
    bsh = nc.dram_tensor("b_shard", [P, 1], F32, kind="ExternalInput")
    idn = nc.dram_tensor("identity", [P, P], F32, kind="ExternalInput")
    iot = nc.dram_tensor("iota_row", [1, P], F32, kind="ExternalInput")
    pfd = nc.dram_tensor("pf128", [P, 1], F32, kind="ExternalInput")
    out = nc.dram_tensor("out_shard", [P, 1], F32, kind="ExternalOutput")

    q_loc = nc.dram_tensor("q_loc", [1, DIM], F32)
    idx_loc = nc.dram_tensor("idx_loc", [1, 1], U32)
    wp8_loc = nc.dram_tensor("wp8_loc", [1, 1], U32)
    dots_loc = nc.dram_tensor("dots_loc", [1, ROWS_PC], F32)
    cand_loc = nc.dram_tensor("cand_loc", [2, CW], F32)
    cand_shr = nc.dram_tensor("cand_shr", [N_CORES, CW], F32, addr_space="Shared")
    warm_loc = nc.dram_tensor("warm_loc", [1, 1], F32)
    warm_shr = nc.dram_tensor("warm_shr", [1, 1], F32, addr_space="Shared")

    groups = [list(range(N_CORES))]

    with tile.TileContext(nc) as tc, ExitStack() as ctx:
        const1 = ctx.enter_context(tc.tile_pool(name="const", bufs=1))
        small = ctx.enter_context(tc.tile_pool(name="small", bufs=1))

        # ---------- warm-up collective (pays barrier + mesh establishment
        # off the critical path, concurrent with the main loop) ------------
        warm = small.tile([1, 1], F32)
        nc.vector.memset(warm, 0.0)
        nc.sync.dma_start(out=warm_loc[:], in_=warm[:])
        nc.gpsimd.collective_compute(
            "AllReduce",
            OP.add,
            replica_groups=groups,
            ins=[warm_loc[:]],
            outs=[warm_shr[:]],
        )

        # consts + decoder weights on the scalar ring (prefetch early)
        idn_sb = const1.tile([P, P], F32)
        nc.scalar.dma_start(out=idn_sb[:], in_=idn[:])
        iot_sb = const1.tile([1, P], F32)
        nc.scalar.dma_start(out=iot_sb[:], in_=iot[:])
        pf_sb = const1.tile([P, 1], F32)
        nc.scalar.dma_start(out=pf_sb[:], in_=pfd[:])
        w_sb = const1.tile([P, DIM], F32)
        nc.scalar.dma_start(out=w_sb[:], in_=wsh[:])
        b_sb = small.tile([P, 1], F32)
        nc.scalar.dma_start(out=b_sb[:], in_=bsh[:])

        # ---------- Phase Q: q = column sums of the replicated query -------
        # fp8 query tiles go FIRST on the sync ring so q is ready early; the
        # bank stream queues behind them. Column sums via DoubleRow
        # ones-matmuls (256 rows contracted per matmul, fp32 PSUM accum).
        ones_dr = const1.tile([P, 2, 16], F8)
        nc.vector.memset(ones_dr, 1.0)
        q_sb = small.tile([1, DIM], F32)
        NQT = SEQ // 256  # 8 row-block tiles
        qw_pad = const1.tile([P, 2 * NCP, 16], F8)
        with tc.tile_pool(name="qtp", bufs=6) as qtp, tc.tile_pool(
            name="qps", bufs=1, space="PSUM"
        ) as qps:
            q_ps = [
                qps.tile([1, 512], F32, name=f"q_ps{h}", tag=f"q_ps{h}")
                for h in range(2)
            ]
            for a in range(NQT):
                qt = qtp.tile([P, 2, DIM], F8, tag="qt")
                nc.sync.dma_start(out=qt[:], in_=qry8[a])
                for h in range(2):
                    nc.tensor.matmul(
                        out=q_ps[h][:],
                        lhsT=ones_dr[:, :, 0:1],
                        rhs=qt[:, :, h * 512 : (h + 1) * 512],
                        start=(a == 0),
                        stop=(a == NQT - 1),
                        perf_mode=PM.DoubleRow,
                    )
            for h in range(2):
                nc.vector.tensor_copy(
                    out=q_sb[:, h * 512 : (h + 1) * 512], in_=q_ps[h][:]
                )

        # transpose q onto partitions with the PE, then downscale into the
        # padded fp8 DoubleRow weight layout (k-plane stride 16B)
        one_1 = small.tile([1, 1], F32)
        nc.vector.memset(one_1, 1.0)
        qf_t = small.tile([P, 2 * NCP], F32)
        with tc.tile_pool(name="tps", bufs=1, space="PSUM") as tps:
            for c in range(2 * NCP):
                tq_ps = tps.tile([P, 1], F32, name=f"tq{c}", tag=f"tq{c}")
                nc.tensor.matmul(
                    out=tq_ps[:],
                    lhsT=q_sb[0:1, c * P : (c + 1) * P],
                    rhs=one_1[:],
                    start=True,
                    stop=True,
                )
                nc.vector.tensor_copy(out=qf_t[:, c : c + 1], in_=tq_ps[:])
        nc.vector.tensor_scalar_mul(
            qw_pad[:, :, 0:1], qf_t[:].rearrange("p (t u) -> p t u", u=1), Q8_SCALE
        )

        # fp32 q broadcast for the rerank (prefetched during the main loop)
        nc.scalar.dma_start(out=q_loc[:], in_=q_sb[:])
        qb32 = const1.tile([P, DIM], F32)
        nc.scalar.dma_start(out=qb32[:], in_=_bcast_ap(q_loc, 0, P, DIM))
        # ||q||^2 and threshold on every partition (from the broadcast q)
        dumA = small.tile([P, 1], F32)
        qn2_128 = small.tile([P, 1], F32)
        nc.scalar.activation(
            out=dumA[:].broadcast_to([P, DIM]),
            in_=qb32[:],
            func=AF.Square,
            accum_out=qn2_128[:],
        )
        thr128 = small.tile([P, 1], F32)
        nc.vector.tensor_scalar_mul(thr128[:], qn2_128[:], THR2)

        # ---------- Phase MAIN: fp8 DoubleRow matvec over the bank ---------
        dots_sb = const1.tile([1, NB, BS], F32)
        rings = [nc.sync, nc.scalar, nc.gpsimd]
        with tc.tile_pool(name="work", bufs=12) as work, tc.tile_pool(
            name="dps", bufs=1, space="PSUM"
        ) as dps_pool:
            for jb in range(NJ):
                tiles = []
                for cp in range(NCP):
                    bt = work.tile([P, 2, JROWS], F8, tag="bt")
                    ring = rings[(jb * NCP + cp) % 3]
                    ring.dma_start(
                        out=bt[:], in_=bank8[cp][:, :, jb * JROWS : (jb + 1) * JROWS]
                    )
                    tiles.append(bt)
                d_ps = [
                    dps_pool.tile([1, BS], F32, name=f"dps{b}_{jb}", tag=f"dps{b}")
                    for b in range(8)
                ]
                for cp in range(NCP):
                    for b in range(8):
                        nc.tensor.matmul(
                            out=d_ps[b][:],
                            lhsT=qw_pad[:, 2 * cp : 2 * cp + 2, 0:1],
                            rhs=tiles[cp][:, :, b * BS : (b + 1) * BS],
                            start=(cp == 0),
                            stop=(cp == NCP - 1),
                            perf_mode=PM.DoubleRow,
                        )
                for b in range(8):
                    dst = dots_sb[0:1, jb * 8 + b, :]
                    if b % 2 == 0:
                        nc.vector.tensor_copy(out=dst, in_=d_ps[b][:])
                    else:
                        nc.scalar.activation(out=dst, in_=d_ps[b][:], func=AF.Copy)
                nc.sync.dma_start(
                    out=bass.AP(
                        tensor=dots_loc,
                        offset=jb * 8 * BS,
                        ap=[[0, 1], [1, 8 * BS]],
                    ),
                    in_=dots_sb[0:1, jb * 8 : (jb + 1) * 8, :],
                )

        # ---------- Phase FILTER: per-partition argmax over fp8 dots -------
        dview = small.tile([P, P], F32)
        nc.sync.dma_start(
            out=dview[:], in_=bass.AP(tensor=dots_loc, offset=0, ap=[[P, P], [1, P]])
        )
        v8 = small.tile([P, 8], F32)
        i8 = small.tile([P, 8], U32)
        nc.vector.max_with_indices(v8[:], i8[:], dview[:])
        fi = small.tile([P, 1], F32)
        nc.vector.tensor_copy(out=fi[:], in_=i8[:, 0:1])  # u32 -> f32
        rowf = small.tile([P, 1], F32)
        nc.vector.tensor_tensor(out=rowf[:], in0=pf_sb[:], in1=fi[:], op=OP.add)
        rowu = small.tile([P, 1], U32)
        nc.vector.tensor_copy(out=rowu[:], in_=rowf[:])  # f32 -> u32
        rows128 = small.tile([P, DIM], F32)
        nc.gpsimd.indirect_dma_start(
            out=rows128[:],
            out_offset=None,
            in_=bankf[:],
            in_offset=bass.IndirectOffsetOnAxis(ap=rowu[:, 0:1], axis=0),
        )

        # ---------- Phase RERANK: exact fp32 cosine scores -----------------
        dumV = small.tile([P, 1], F32)
        S = small.tile([P, 1], F32)
        nc.scalar.activation(
            out=dumA[:].broadcast_to([P, DIM]),
            in_=rows128[:],
            func=AF.Square,
            accum_out=S[:],
        )
        D = small.tile([P, 1], F32)
        nc.vector.scalar_tensor_tensor(
            out=dumV[:].broadcast_to([P, DIM]),
            in0=rows128[:],
            scalar=1.0,
            in1=qb32[:],
            op0=OP.mult,
            op1=OP.mult,
            accum_out=D[:],
        )
        Sg = small.tile([P, 1], F32)
        nc.vector.tensor_scalar_add(Sg[:], S[:], 1e-20)
        Rcp = small.tile([P, 1], F32)
        nc.vector.reciprocal(Rcp[:], Sg[:])
        Dn = small.tile([P, 1], F32)
        nc.vector.tensor_scalar_mul(Dn[:], D[:], -1.0)
        Ab = small.tile([P, 1], F32)
        nc.vector.tensor_tensor(out=Ab[:], in0=D[:], in1=Dn[:], op=OP.max)
        DA = small.tile([P, 1], F32)
        nc.vector.tensor_tensor(out=DA[:], in0=D[:], in1=Ab[:], op=OP.mult)
        Fs = small.tile([P, 1], F32)
        nc.vector.tensor_tensor(out=Fs[:], in0=DA[:], in1=Rcp[:], op=OP.mult)

        # local argmax across the 128 candidates (partitions)
        with tc.tile_pool(name="fps", bufs=1, space="PSUM") as fps:
            tv_ps = fps.tile([1, P], F32, tag="tv")
            nc.tensor.transpose(out=tv_ps[:], in_=Fs[:], identity=idn_sb[:])
            tc_ps = fps.tile([1, P], F32, tag="tc")
            nc.tensor.transpose(out=tc_ps[:], in_=rowf[:], identity=idn_sb[:])
            Tv = small.tile([1, P], F32)
            nc.vector.tensor_copy(out=Tv[:], in_=tv_ps[:])
            gv8 = small.tile([1, 8], F32)
            gp8 = small.tile([1, 8], U32)
            nc.vector.max_with_indices(gv8[:], gp8[:], Tv[:])
            wcolf = small.tile([1, 1], F32)
            nc.vector.tensor_copy(out=wcolf[:], in_=gp8[0:1, 0:1])
            oh = small.tile([1, P], F32)
            nc.vector.tensor_scalar(oh[:], iot_sb[:], wcolf[0:1, 0:1], None, OP.is_equal)
            ohc = small.tile([1, P], F32)
            nc.vector.tensor_tensor(out=ohc[:], in0=oh[:], in1=tc_ps[:], op=OP.mult)
            wrowf = small.tile([1, 1], F32)
            nc.vector.reduce_sum(out=wrowf[:], in_=ohc[:], axis=AX.X)
            # replicate the winner row index onto 2 partitions via a rank-1
            # PE matmul (no DRAM roundtrip), then gather the row straight
            # into the DRAM-resident candidate record
            ones_row = const1.tile([1, P], F32)
            nc.vector.memset(ones_row, 1.0)
            ix2_ps = fps.tile([2, 1], F32, tag="ix2")
            nc.tensor.matmul(
                out=ix2_ps[:],
                lhsT=ones_row[0:1, 0:2],
                rhs=wrowf[:],
                start=True,
                stop=True,
            )
            idx2 = small.tile([2, 1], U32)
            nc.vector.tensor_copy(out=idx2[:], in_=ix2_ps[:])  # f32 -> u32
            cnd = small.tile([2, CW], F32)
            nc.vector.tensor_copy(out=cnd[0:1, 0:1], in_=gv8[0:1, 0:1])
            nc.gpsimd.indirect_dma_start(
                out=cnd[:, 2:CW],
                out_offset=None,
                in_=bankf[:],
                in_offset=bass.IndirectOffsetOnAxis(ap=idx2[:, 0:1], axis=0),
            )
            nc.scalar.dma_start(out=cand_loc[0:1, :], in_=cnd[0:1, :])
            nc.gpsimd.collective_compute(
                "AllGather",
                OP.bypass,
                replica_groups=groups,
                ins=[cand_loc[0:1, :]],
                outs=[cand_shr[:]],
            )

            # ---------- Phase FINAL: global winner, broadcast, decode ------
            sc8 = small.tile([1, N_CORES, 1], F32)
            nc.scalar.dma_start(
                out=sc8[:],
                in_=bass.AP(
                    tensor=cand_shr, offset=0, ap=[[0, 1], [CW, N_CORES], [1, 1]]
                ),
            )
            g8v = small.tile([1, 8], F32)
            g8i = small.tile([1, 8], U32)
            nc.vector.max_with_indices(
                g8v[:], g8i[:], sc8[:].rearrange("o c u -> o (c u)")
            )
            wpf = small.tile([1, 1], F32)
            nc.vector.tensor_copy(out=wpf[:], in_=g8i[0:1, 0:1])  # u32 -> f32
            wp_ps = fps.tile([P, 1], F32, tag="wp")
            nc.tensor.matmul(
                out=wp_ps[:], lhsT=ones_row[:], rhs=wpf[:], start=True, stop=True
            )
            wp128 = small.tile([P, 1], U32)
            nc.vector.tensor_copy(out=wp128[:], in_=wp_ps[:])  # f32 -> u32
            win128 = small.tile([P, CW], F32)
            nc.gpsimd.indirect_dma_start(
                out=win128[:],
                out_offset=None,
                in_=cand_shr[:],
                in_offset=bass.IndirectOffsetOnAxis(ap=wp128[:, 0:1], axis=0),
            )
            # decode: out = ind * (w_shard . row + b_shard), all per-partition
            dec = small.tile([P, 1], F32)
            nc.vector.scalar_tensor_tensor(
                out=dumV[:].broadcast_to([P, DIM]),
                in0=w_sb[:],
                scalar=1.0,
                in1=win128[:, 2:CW],
                op0=OP.mult,
                op1=OP.mult,
                accum_out=dec[:],
            )
            ind128 = small.tile([P, 1], F32)
            nc.vector.tensor_tensor(
                out=ind128[:], in0=win128[:, 0:1], in1=thr128[:], op=OP.is_gt
            )
            dsum = small.tile([P, 1], F32)
            nc.vector.tensor_tensor(out=dsum[:], in0=dec[:], in1=b_sb[:], op=OP.add)
            o_sb = small.tile([P, 1], F32)
            nc.vector.tensor_tensor(out=o_sb[:], in0=dsum[:], in1=ind128[:], op=OP.mult)
            nc.scalar.dma_start(out=out[:], in_=o_sb[:])

    _split_multi_waits(nc)
    return nc


def make_in_maps(query, bank, w_dec, b_dec):
    # query_f8[a, p, t, d] = query[256 a + 128 t + p, d]
    q_f8 = np.ascontiguousarray(
        np.asarray(query, dtype=np.float32)
        .astype(ml_dtypes.float8_e4m3)
        .reshape(SEQ // 256, 2, P, DIM)
        .transpose(0, 2, 1, 3)
    )
    identity = np.eye(P, dtype=np.float32)
    iota_row = np.arange(P, dtype=np.float32).reshape(1, P)
    pf128 = (np.arange(P, dtype=np.float32) * P).reshape(P, 1)
    in_maps = []
    for c in range(N_CORES):
        shard = np.ascontiguousarray(bank[c * ROWS_PC : (c + 1) * ROWS_PC]).astype(
            np.float32
        )
        b8 = shard.astype(ml_dtypes.float8_e4m3)
        # bank_t8[cp, p, t, j] = shard[j, 256 cp + 128 t + p]
        bank_t8 = np.ascontiguousarray(
            b8.T.reshape(NCP, 2, P, ROWS_PC).transpose(0, 2, 1, 3)
        )
        in_maps.append(
            {
                "bank_t8": bank_t8,
                "bank_f32": shard,
                "query_f8": q_f8,
                "w_shard": np.ascontiguousarray(
                    w_dec[c * WROWS_PC : (c + 1) * WROWS_PC], dtype=np.float32
                ),
                "b_shard": np.ascontiguousarray(
                    b_dec[c * WROWS_PC : (c + 1) * WROWS_PC], dtype=np.float32
                ).reshape(WROWS_PC, 1),
                "identity": identity,
                "iota_row": iota_row,
                "pf128": pf128,
            }
        )
    return in_maps


_NC_CACHE = {}


def _get_nc():
    if "nc" not in _NC_CACHE:
        _NC_CACHE["nc"] = build_kernel()
    return _NC_CACHE["nc"]


def run(query, bank, w_dec, b_dec, trace=False):
    nc = _get_nc()
    in_maps = make_in_maps(query, bank, w_dec, b_dec)
    if trace:
        # warm-up execution: loads the NEFF on all cores so the traced run
        # isn't skewed by first-launch dispatch staggering
        run_bass_kernel_spmd(nc, in_maps, list(range(N_CORES)), trace=False)
    res = run_bass_kernel_spmd(nc, in_maps, list(range(N_CORES)), trace=trace)
    outp = np.concatenate(
        [res.results[c]["out_shard"][:, 0] for c in range(N_CORES)]
    ).astype(np.float32)
    return outp, res


def kernel(query, bank, w_dec, b_dec):
    outp, _ = run(query, bank, w_dec, b_dec)
    return outp


# revision 49
# speedup vs baseline: 1.0287x; 1.0287x over previous
"""Trainium2 Bass kernel for nn_BiologicalMemory (retrieval_knn).

Computes: q = mean(query, axis=0); sims = cosine(bank, q); i* = argmax(sims);
out = (sims[i*] > 0.65) ? bank[i*] @ w_dec.T + b_dec : zeros.

Strategy (8 NeuronCores, SPMD), filter-then-rerank:
  - bank rows sharded 16384/core. The similarity SEARCH streams an fp8-e4m3
    host-transposed copy of the bank (16 MB/core, 4 KB DMA lines) through the
    PE as DoubleRow matvecs against q (fp8): 512-row dot blocks accumulate in
    PSUM over 4 chunk-pair matmuls (256-dim contraction each).
  - q = column sums of the replicated bf16 query, accumulated on the PE with
    a ones-vector lhsT (fp32 PSUM accumulation).
  - block dots are copied to SBUF, DMA-round-tripped into a [128,128] layout,
    and each partition's top row (by fp8 dot) becomes a rerank candidate.
  - rerank: indirect-gather the 128 candidate rows in fp32, compute exact
    dots vs fp32 q and exact squared norms, score f = dot*|dot|/||x||^2
    (monotone in cosine), and pick the local winner exactly.
  - AllGather 8 candidate records [score, 1.0, row(fp32)]; winner selected by
    score; its row (and the threshold indicator, via the 1.0 marker column)
    is broadcast across partitions with a rank-1 PE matmul; decode is exact
    fp32: out = w_shard . (ind*row) + ind*b_shard per core (128 features).
"""

import os
import sys

import numpy as np

for _p in ("/opt/trn_rl_repo",):
    if os.path.isdir(_p) and _p not in sys.path:
        sys.path.insert(0, _p)

from contextlib import ExitStack

import ml_dtypes
import concourse.bass as bass
import concourse.tile as tile
from concourse import mybir
from concourse.bass_utils import run_bass_kernel_spmd

N_CORES = 8
SEQ, DIM, N_MEM = 2048, 1024, 131072
ROWS_PC = N_MEM // N_CORES  # 16384 bank rows per core
WROWS_PC = DIM // N_CORES  # 128 decoder rows per core
P = 128
NCP = 4  # chunk-pairs (256 dims contracted per DoubleRow matmul)
NB = 32  # 512-row dot blocks per core
BS = 512  # rows per block
NJ = 4  # jb groups (8 blocks each); tile free = 4096 rows
JROWS = ROWS_PC // NJ  # 4096
Q_TILES = SEQ // P  # 16
THR2 = 0.65 * 0.65
Q8_SCALE = 1.0 / 16.0
CW = 2 + DIM  # record: [score, marker=1.0, row...]

F32 = mybir.dt.float32
BF16 = mybir.dt.bfloat16
F8 = mybir.dt.float8e4
U32 = mybir.dt.uint32
AX = mybir.AxisListType
OP = mybir.AluOpType
AF = mybir.ActivationFunctionType
PM = mybir.MatmulPerfMode

_MAX_WAITS = 1


def _split_multi_waits(nc, max_waits=_MAX_WAITS):
    """This walrus build accepts at most one sync-wait per instruction.
    Hoist extra waits onto injected same-engine Drain instructions placed
    immediately before the over-subscribed instruction."""
    counter = 0
    for f in nc.m.functions:
        for bb in f.blocks:
            insts = list(bb.instructions)
            out = []
            changed = False
            for inst in insts:
                si = getattr(inst, "sync_info", None)
                waits = list(si.on_wait) if (si is not None and si.on_wait) else []
                if len(waits) > max_waits:
                    changed = True
                    extra, keep = waits[:-max_waits], waits[-max_waits:]
                    for w in extra:
                        counter += 1
                        d = mybir.InstDrain(name=f"waitsplit-{counter}")
                        d.engine = inst.engine
                        d.sync_info = mybir.SyncInfo(on_wait=[w], on_update=[])
                        out.append(d)
                    inst.sync_info = mybir.SyncInfo(
                        on_wait=keep, on_update=list(si.on_update or [])
                    )
                out.append(inst)
            if changed:
                bb.instructions = out


def _bcast_ap(handle, offset, nparts, nfree):
    """DRAM AP that replicates a contiguous [nfree] region across nparts."""
    return bass.AP(tensor=handle, offset=offset, ap=[[0, nparts], [1, nfree]])


def build_kernel():
    nc = bass.Bass(num_devices=N_CORES)

    bank8 = nc.dram_tensor("bank_t8", [NCP, P, 2, ROWS_PC], F8, kind="ExternalInput")
    bankb = nc.dram_tensor("bank_bf", [ROWS_PC, DIM], BF16, kind="ExternalInput")
    qry8 = nc.dram_tensor("query_f8", [SEQ // 256, P, 2, DIM], F8, kind="ExternalInput")
    wsh = nc.dram_tensor("w_shard", [P, DIM], F32, kind="ExternalInput")
    bsh = nc.dram_tensor("b_shard", [P, 1], F32, kind="ExternalInput")
    idn = nc.dram_tensor("identity", [P, P], F32, kind="ExternalInput")
    iot = nc.dram_tensor("iota_row", [1, P], F32, kind="ExternalInput")
    pfd = nc.dram_tensor("pf128", [P, 1], F32, kind="ExternalInput")
    out = nc.dram_tensor("out_shard", [P, 1], F32, kind="ExternalOutput")

    qb_loc = nc.dram_tensor("qb_loc", [1, DIM], BF16)
    dots_loc = nc.dram_tensor("dots_loc", [1, ROWS_PC], F32)
    cand_loc = nc.dram_tensor("cand_loc", [2, CW], F32)
    cand_shr = nc.dram_tensor("cand_shr", [N_CORES, CW], F32, addr_space="Shared")
    warm_loc = nc.dram_tensor("warm_loc", [1, 1], F32)
    warm_shr = nc.dram_tensor("warm_shr", [1, 1], F32, addr_space="Shared")

    groups = [list(range(N_CORES))]

    with tile.TileContext(nc) as tc, ExitStack() as ctx:
        const1 = ctx.enter_context(tc.tile_pool(name="const", bufs=1))
        small = ctx.enter_context(tc.tile_pool(name="small", bufs=1))

        # ---------- warm-up collective (pays barrier + mesh establishment
        # off the critical path, concurrent with the main loop) ------------
        warm = small.tile([1, 1], F32)
        nc.vector.memset(warm, 0.0)
        nc.sync.dma_start(out=warm_loc[:], in_=warm[:])
        nc.gpsimd.collective_compute(
            "AllReduce",
            OP.add,
            replica_groups=groups,
            ins=[warm_loc[:]],
            outs=[warm_shr[:]],
        )

        # consts + decoder weights on the scalar ring (prefetch early)
        idn_sb = const1.tile([P, P], F32)
        nc.scalar.dma_start(out=idn_sb[:], in_=idn[:])
        iot_sb = const1.tile([1, P], F32)
        nc.scalar.dma_start(out=iot_sb[:], in_=iot[:])
        pf_sb = const1.tile([P, 1], F32)
        nc.scalar.dma_start(out=pf_sb[:], in_=pfd[:])
        w_sb = const1.tile([P, DIM], F32)
        nc.scalar.dma_start(out=w_sb[:], in_=wsh[:])
        b_sb = small.tile([P, 1], F32)
        nc.scalar.dma_start(out=b_sb[:], in_=bsh[:])

        # ---------- Phase Q: q = column sums of the replicated query -------
        # fp8 query tiles go FIRST on the sync ring so q is ready early; the
        # bank stream queues behind them. Column sums via DoubleRow
        # ones-matmuls (256 rows contracted per matmul, fp32 PSUM accum).
        ones_dr = const1.tile([P, 2, 16], F8)
        nc.vector.memset(ones_dr, 1.0)
        q_sb = small.tile([1, DIM], F32)
        NQT = SEQ // 256  # 8 row-block tiles
        qw_pad = const1.tile([P, 2 * NCP, 16], F8)
        with tc.tile_pool(name="qtp", bufs=6) as qtp, tc.tile_pool(
            name="qps", bufs=1, space="PSUM"
        ) as qps:
            q_ps = [
                qps.tile([1, 512], F32, name=f"q_ps{h}", tag=f"q_ps{h}")
                for h in range(2)
            ]
            for a in range(NQT):
                qt = qtp.tile([P, 2, DIM], F8, tag="qt")
                nc.sync.dma_start(out=qt[:], in_=qry8[a])
                for h in range(2):
                    nc.tensor.matmul(
                        out=q_ps[h][:],
                        lhsT=ones_dr[:, :, 0:1],
                        rhs=qt[:, :, h * 512 : (h + 1) * 512],
                        start=(a == 0),
                        stop=(a == NQT - 1),
                        perf_mode=PM.DoubleRow,
                    )
            for h in range(2):
                nc.vector.tensor_copy(
                    out=q_sb[:, h * 512 : (h + 1) * 512], in_=q_ps[h][:]
                )

        # transpose q onto partitions with the PE, then downscale into the
        # padded fp8 DoubleRow weight layout (k-plane stride 16B)
        one_1 = small.tile([1, 1], F32)
        nc.vector.memset(one_1, 1.0)
        qf_t = small.tile([P, 2 * NCP], F32)
        with tc.tile_pool(name="tps", bufs=1, space="PSUM") as tps:
            for c in range(2 * NCP):
                tq_ps = tps.tile([P, 1], F32, name=f"tq{c}", tag=f"tq{c}")
                nc.tensor.matmul(
                    out=tq_ps[:],
                    lhsT=q_sb[0:1, c * P : (c + 1) * P],
                    rhs=one_1[:],
                    start=True,
                    stop=True,
                )
                nc.vector.tensor_copy(out=qf_t[:, c : c + 1], in_=tq_ps[:])
        nc.vector.tensor_scalar_mul(
            qw_pad[:, :, 0:1], qf_t[:].rearrange("p (t u) -> p t u", u=1), Q8_SCALE
        )

        # bf16 q broadcast for the rerank (prefetched during the main loop)
        qbf = small.tile([1, DIM], BF16)
        nc.vector.tensor_copy(out=qbf[:], in_=q_sb[:])
        nc.scalar.dma_start(out=qb_loc[:], in_=qbf[:])
        qb32 = const1.tile([P, DIM], BF16)
        nc.scalar.dma_start(out=qb32[:], in_=_bcast_ap(qb_loc, 0, P, DIM))
        # ||q||^2 and threshold on every partition (from the broadcast q)
        dumA = small.tile([P, 1], F32)
        qn2_128 = small.tile([P, 1], F32)
        nc.scalar.activation(
            out=dumA[:].broadcast_to([P, DIM]),
            in_=qb32[:],
            func=AF.Square,
            accum_out=qn2_128[:],
        )
        thr128 = small.tile([P, 1], F32)
        nc.vector.tensor_scalar_mul(thr128[:], qn2_128[:], THR2)

        # ---------- Phase MAIN: fp8 DoubleRow matvec over the bank ---------
        dots_sb = const1.tile([1, NB, BS], F32)
        rings = [nc.sync, nc.scalar, nc.gpsimd]
        with tc.tile_pool(name="work", bufs=12) as work, tc.tile_pool(
            name="dps", bufs=1, space="PSUM"
        ) as dps_pool:
            for jb in range(NJ):
                tiles = []
                for cp in range(NCP):
                    bt = work.tile([P, 2, JROWS], F8, tag="bt")
                    ring = rings[(jb * NCP + cp) % 3]
                    ring.dma_start(
                        out=bt[:], in_=bank8[cp][:, :, jb * JROWS : (jb + 1) * JROWS]
                    )
                    tiles.append(bt)
                d_ps = [
                    dps_pool.tile([1, BS], F32, name=f"dps{b}_{jb}", tag=f"dps{b}")
                    for b in range(8)
                ]
                for cp in range(NCP):
                    for b in range(8):
                        nc.tensor.matmul(
                            out=d_ps[b][:],
                            lhsT=qw_pad[:, 2 * cp : 2 * cp + 2, 0:1],
                            rhs=tiles[cp][:, :, b * BS : (b + 1) * BS],
                            start=(cp == 0),
                            stop=(cp == NCP - 1),
                            perf_mode=PM.DoubleRow,
                        )
                for b in range(8):
                    dst = dots_sb[0:1, jb * 8 + b, :]
                    if b % 2 == 0:
                        nc.vector.tensor_copy(out=dst, in_=d_ps[b][:])
                    else:
                        nc.scalar.activation(out=dst, in_=d_ps[b][:], func=AF.Copy)
                nc.sync.dma_start(
                    out=bass.AP(
                        tensor=dots_loc,
                        offset=jb * 8 * BS,
                        ap=[[0, 1], [1, 8 * BS]],
                    ),
                    in_=dots_sb[0:1, jb * 8 : (jb + 1) * 8, :],
                )

        # ---------- Phase FILTER: per-partition argmax over fp8 dots -------
        dview = small.tile([P, P], F32)
        nc.sync.dma_start(
            out=dview[:], in_=bass.AP(tensor=dots_loc, offset=0, ap=[[P, P], [1, P]])
        )
        v8 = small.tile([P, 8], F32)
        i8 = small.tile([P, 8], U32)
        nc.vector.max_with_indices(v8[:], i8[:], dview[:])
        fi = small.tile([P, 1], F32)
        nc.vector.tensor_copy(out=fi[:], in_=i8[:, 0:1])  # u32 -> f32
        rowf = small.tile([P, 1], F32)
        nc.vector.tensor_tensor(out=rowf[:], in0=pf_sb[:], in1=fi[:], op=OP.add)
        rowu = small.tile([P, 1], U32)
        nc.vector.tensor_copy(out=rowu[:], in_=rowf[:])  # f32 -> u32
        rows128 = small.tile([P, DIM], BF16)
        nc.gpsimd.indirect_dma_start(
            out=rows128[:],
            out_offset=None,
            in_=bankb[:],
            in_offset=bass.IndirectOffsetOnAxis(ap=rowu[:, 0:1], axis=0),
        )

        # ---------- Phase RERANK: exact cosine scores (bf16 in, f32 acc) ---
        dumV = small.tile([P, 1], F32)
        dumB = small.tile([P, 1], BF16)
        S = small.tile([P, 1], F32)
        nc.scalar.activation(
            out=dumA[:].broadcast_to([P, DIM]),
            in_=rows128[:],
            func=AF.Square,
            accum_out=S[:],
        )
        D = small.tile([P, 1], F32)
        nc.vector.scalar_tensor_tensor(
            out=dumB[:].broadcast_to([P, DIM]),
            in0=rows128[:],
            scalar=1.0,
            in1=qb32[:],
            op0=OP.mult,
            op1=OP.mult,
            accum_out=D[:],
        )
        Sg = small.tile([P, 1], F32)
        nc.vector.tensor_scalar_add(Sg[:], S[:], 1e-20)
        Rcp = small.tile([P, 1], F32)
        nc.vector.reciprocal(Rcp[:], Sg[:])
        Dn = small.tile([P, 1], F32)
        nc.vector.tensor_scalar_mul(Dn[:], D[:], -1.0)
        Ab = small.tile([P, 1], F32)
        nc.vector.tensor_tensor(out=Ab[:], in0=D[:], in1=Dn[:], op=OP.max)
        DA = small.tile([P, 1], F32)
        nc.vector.tensor_tensor(out=DA[:], in0=D[:], in1=Ab[:], op=OP.mult)
        Fs = small.tile([P, 1], F32)
        nc.vector.tensor_tensor(out=Fs[:], in0=DA[:], in1=Rcp[:], op=OP.mult)

        # local argmax across the 128 candidates (partitions)
        with tc.tile_pool(name="fps", bufs=1, space="PSUM") as fps:
            tv_ps = fps.tile([1, P], F32, tag="tv")
            nc.tensor.transpose(out=tv_ps[:], in_=Fs[:], identity=idn_sb[:])
            tc_ps = fps.tile([1, P], F32, tag="tc")
            nc.tensor.transpose(out=tc_ps[:], in_=rowf[:], identity=idn_sb[:])
            Tv = small.tile([1, P], F32)
            nc.vector.tensor_copy(out=Tv[:], in_=tv_ps[:])
            gv8 = small.tile([1, 8], F32)
            gp8 = small.tile([1, 8], U32)
            nc.vector.max_with_indices(gv8[:], gp8[:], Tv[:])
            wcolf = small.tile([1, 1], F32)
            nc.vector.tensor_copy(out=wcolf[:], in_=gp8[0:1, 0:1])
            oh = small.tile([1, P], F32)
            nc.vector.tensor_scalar(oh[:], iot_sb[:], wcolf[0:1, 0:1], None, OP.is_equal)
            ohc = small.tile([1, P], F32)
            nc.vector.tensor_tensor(out=ohc[:], in0=oh[:], in1=tc_ps[:], op=OP.mult)
            wrowf = small.tile([1, 1], F32)
            nc.vector.reduce_sum(out=wrowf[:], in_=ohc[:], axis=AX.X)
            # winner row extracted from rows128 with a PE mask-matmul:
            # transpose the onehot onto partitions, then contract
            ones_row = const1.tile([1, P], F32)
            nc.vector.memset(ones_row, 1.0)
            ohT_ps = fps.tile([P, 1], F32, tag="ohT")
            nc.tensor.matmul(
                out=ohT_ps[:], lhsT=oh[:], rhs=one_1[:], start=True, stop=True
            )
            ohT_bf = small.tile([P, 1], BF16)
            nc.vector.tensor_copy(out=ohT_bf[:], in_=ohT_ps[:])
            cnd = small.tile([1, CW], F32)
            nc.vector.tensor_copy(out=cnd[0:1, 0:1], in_=gv8[0:1, 0:1])
            for h in range(2):
                row_ps = fps.tile([1, 512], F32, name=f"rowp{h}", tag=f"rowp{h}")
                nc.tensor.matmul(
                    out=row_ps[:],
                    lhsT=ohT_bf[:],
                    rhs=rows128[:, h * 512 : (h + 1) * 512],
                    start=True,
                    stop=True,
                )
                nc.scalar.activation(
                    out=cnd[:, 2 + h * 512 : 2 + (h + 1) * 512],
                    in_=row_ps[:],
                    func=AF.Copy,
                )
            nc.scalar.dma_start(out=cand_loc[0:1, :], in_=cnd[0:1, :])

        with tc.tile_pool(name="fps2", bufs=1, space="PSUM") as fps:
            nc.gpsimd.collective_compute(
                "AllGather",
                OP.bypass,
                replica_groups=groups,
                ins=[cand_loc[0:1, :]],
                outs=[cand_shr[:]],
            )

            # ---------- Phase FINAL: global winner, broadcast, decode ------
            sc8 = small.tile([1, N_CORES, 1], F32)
            nc.scalar.dma_start(
                out=sc8[:],
                in_=bass.AP(
                    tensor=cand_shr, offset=0, ap=[[0, 1], [CW, N_CORES], [1, 1]]
                ),
            )
            g8v = small.tile([1, 8], F32)
            g8i = small.tile([1, 8], U32)
            nc.vector.max_with_indices(
                g8v[:], g8i[:], sc8[:].rearrange("o c u -> o (c u)")
            )
            wpf = small.tile([1, 1], F32)
            nc.vector.tensor_copy(out=wpf[:], in_=g8i[0:1, 0:1])  # u32 -> f32
            wp_ps = fps.tile([2, 1], F32, tag="wp")
            nc.tensor.matmul(
                out=wp_ps[:], lhsT=ones_row[0:1, 0:2], rhs=wpf[:], start=True, stop=True
            )
            wp2 = small.tile([2, 1], U32)
            nc.vector.tensor_copy(out=wp2[:], in_=wp_ps[:])  # f32 -> u32
            win2 = small.tile([2, CW], F32)
            nc.gpsimd.indirect_dma_start(
                out=win2[:],
                out_offset=None,
                in_=cand_shr[:],
                in_offset=bass.IndirectOffsetOnAxis(ap=wp2[:, 0:1], axis=0),
            )
            # rank-1 PE broadcast of [score, row] onto all 128 partitions
            sc_ps = fps.tile([P, 1], F32, tag="scp")
            nc.tensor.matmul(
                out=sc_ps[:], lhsT=ones_row[:], rhs=win2[0:1, 0:1], start=True, stop=True
            )
            dec = [small.tile([P, 1], F32, name=f"dec{h}") for h in range(2)]
            for h in range(2):
                bm_ps = fps.tile([P, 512], F32, name=f"bmp{h}", tag=f"bmp{h}")
                nc.tensor.matmul(
                    out=bm_ps[:],
                    lhsT=ones_row[:],
                    rhs=win2[0:1, 2 + h * 512 : 2 + (h + 1) * 512],
                    start=True,
                    stop=True,
                )
                nc.vector.scalar_tensor_tensor(
                    out=dumV[:].broadcast_to([P, 512]),
                    in0=w_sb[:, h * 512 : (h + 1) * 512],
                    scalar=1.0,
                    in1=bm_ps[:],
                    op0=OP.mult,
                    op1=OP.mult,
                    accum_out=dec[h][:],
                )
            ind128 = small.tile([P, 1], F32)
            nc.vector.tensor_tensor(
                out=ind128[:], in0=sc_ps[:], in1=thr128[:], op=OP.is_gt
            )
            dsum = small.tile([P, 1], F32)
            nc.vector.tensor_tensor(out=dsum[:], in0=dec[0][:], in1=dec[1][:], op=OP.add)
            dsb = small.tile([P, 1], F32)
            nc.vector.tensor_tensor(out=dsb[:], in0=dsum[:], in1=b_sb[:], op=OP.add)
            o_sb = small.tile([P, 1], F32)
            nc.vector.tensor_tensor(out=o_sb[:], in0=dsb[:], in1=ind128[:], op=OP.mult)
            nc.scalar.dma_start(out=out[:], in_=o_sb[:])

    _split_multi_waits(nc)
    return nc


def make_in_maps(query, bank, w_dec, b_dec):
    # query_f8[a, p, t, d] = query[256 a + 128 t + p, d]
    q_f8 = np.ascontiguousarray(
        np.asarray(query, dtype=np.float32)
        .astype(ml_dtypes.float8_e4m3)
        .reshape(SEQ // 256, 2, P, DIM)
        .transpose(0, 2, 1, 3)
    )
    identity = np.eye(P, dtype=np.float32)
    iota_row = np.arange(P, dtype=np.float32).reshape(1, P)
    pf128 = (np.arange(P, dtype=np.float32) * P).reshape(P, 1)
    in_maps = []
    for c in range(N_CORES):
        shard = np.ascontiguousarray(bank[c * ROWS_PC : (c + 1) * ROWS_PC]).astype(
            np.float32
        )
        b8 = shard.astype(ml_dtypes.float8_e4m3)
        # bank_t8[cp, p, t, j] = shard[j, 256 cp + 128 t + p]
        bank_t8 = np.ascontiguousarray(
            b8.T.reshape(NCP, 2, P, ROWS_PC).transpose(0, 2, 1, 3)
        )
        in_maps.append(
            {
                "bank_t8": bank_t8,
                "bank_bf": shard.astype(ml_dtypes.bfloat16),
                "query_f8": q_f8,
                "w_shard": np.ascontiguousarray(
                    w_dec[c * WROWS_PC : (c + 1) * WROWS_PC], dtype=np.float32
                ),
                "b_shard": np.ascontiguousarray(
                    b_dec[c * WROWS_PC : (c + 1) * WROWS_PC], dtype=np.float32
                ).reshape(WROWS_PC, 1),
                "identity": identity,
                "iota_row": iota_row,
                "pf128": pf128,
            }
        )
    return in_maps


_NC_CACHE = {}


def _get_nc():
    if "nc" not in _NC_CACHE:
        _NC_CACHE["nc"] = build_kernel()
    return _NC_CACHE["nc"]


def run(query, bank, w_dec, b_dec, trace=False):
    nc = _get_nc()
    in_maps = make_in_maps(query, bank, w_dec, b_dec)
    if trace:
        # warm-up execution: loads the NEFF on all cores so the traced run
        # isn't skewed by first-launch dispatch staggering
        run_bass_kernel_spmd(nc, in_maps, list(range(N_CORES)), trace=False)
    res = run_bass_kernel_spmd(nc, in_maps, list(range(N_CORES)), trace=trace)
    outp = np.concatenate(
        [res.results[c]["out_shard"][:, 0] for c in range(N_CORES)]
    ).astype(np.float32)
    return outp, res


def kernel(query, bank, w_dec, b_dec):
    outp, _ = run(query, bank, w_dec, b_dec)
    return outp
